# revision 14
# baseline (speedup 1.0000x reference)
"""BFP8 block quantize-dequantize for Trainium2 (Bass/Tile), 8-core data parallel.

Structure = the minimal-cross-engine-edge pipeline (load -> DVE reduce/bits/
quant -> GPSIMD dequant -> store), with a hybrid-dtype output that balances
GPSIMD against DMA:
  - GPSIMD dequant costs ~8.6 us/tile writing f32 but ~11.1 us writing bf16
    (Q7 output-conversion cost), while bf16 halves store traffic. Writing the
    first 7/16 of each tile's blocks as f32 and the rest as bf16 puts both
    GPSIMD (~10.0 us) and DMA (~3.44 MiB -> ~10.1 us) just under the DVE+
    sync ceiling, total ~86 MiB HBM/core.
  - Every stored value is EXACT: the f32 part trivially, the bf16 part since
    dequantized values have <= 8 significant bits. Host stitches + upcasts.
  - DVE: block abs-max reduce; exponent bit-math (exact bit tricks on the f32
    exponent field); quant q = sat_int8(round(x * rcp)) via the f32->int8
    output conversion (round-to-nearest-even + clamp for free).
  - Loads + bf16 stores on the SP HWDGE ring; f32 stores on the ACT ring.
Zero/denormal blocks: expb clamps to 0 -> scale 0 -> out exactly 0.
"""
import numpy as np

try:
    import concourse.bacc as bacc
except ImportError:  # pragma: no cover - fallback for bare environments
    import sys
    for _p in ("/opt/trn_rl_repo", "/root/.axon_site/_ro/trn_rl_repo"):
        if _p not in sys.path:
            sys.path.insert(0, _p)
    import concourse.bacc as bacc
import concourse.mybir as mybir
import concourse.tile as tile
from concourse.bass_utils import run_bass_kernel_spmd

N_CORES = 8
P = 128
ROWS, COLS = 4096, 4096
BLK = 16
MBITS_M1 = 7
EXP_MASK = 0x7F800000

TILE_F = 4096
TAPER_N, TAPER_F = 2, 1024
BUFS = 4
A_NUM, A_DEN = 7, 16         # fraction of blocks stored as f32 (GPSIMD f32 path)


def _schedule():
    total_f = ROWS * COLS // P
    end = TAPER_N * TAPER_F
    mid = total_f - 2 * end
    assert mid % TILE_F == 0
    return [TAPER_F] * TAPER_N + [TILE_F] * (mid // TILE_F) + [TAPER_F] * TAPER_N


def _splits():
    sched = _schedule()
    parts = []
    o = o32 = o16 = 0
    for f in sched:
        nb = f // BLK
        ba = nb * A_NUM // A_DEN
        fa = ba * BLK
        parts.append((f, fa, o, o32, o16))
        o += P * f
        o32 += P * fa
        o16 += P * (f - fa)
    return parts, o32, o16


def build(reps=1):
    nc = bacc.Bacc()
    x = nc.dram_tensor("x", [ROWS, COLS], mybir.dt.float32, kind="ExternalInput")
    parts, n32, n16 = _splits()
    out32 = nc.dram_tensor("out32", [n32], mybir.dt.float32, kind="ExternalOutput")
    out16 = nc.dram_tensor("out16", [n16], mybir.dt.bfloat16, kind="ExternalOutput")
    xflat = x[:].rearrange("r c -> (r c)")

    with tile.TileContext(nc) as tc:
        with tc.tile_pool(name="sbuf", bufs=BUFS) as pool:
            for f, fa, o, o32, o16 in [p for _ in range(reps) for p in parts]:
                nb = f // BLK
                ba = fa // BLK
                xt = pool.tile([P, f], mybir.dt.float32, tag="x")
                nc.sync.dma_start(xt[:], xflat[o:o + P * f].rearrange("(p f) -> p f", p=P))
                x3 = xt[:].rearrange("p (b k) -> p b k", k=BLK)

                # block max|x|
                bmax = pool.tile([P, nb], mybir.dt.float32, tag="bmax")
                nc.vector.tensor_reduce(
                    bmax[:], x3, axis=mybir.AxisListType.X,
                    op=mybir.AluOpType.max, apply_absolute_value=True,
                )
                expb = pool.tile([P, nb], mybir.dt.int32, tag="expb")
                nc.vector.tensor_scalar(
                    expb[:], bmax[:].bitcast(mybir.dt.int32),
                    scalar1=EXP_MASK, scalar2=None, op0=mybir.AluOpType.bitwise_and,
                )
                scaleb = pool.tile([P, nb], mybir.dt.int32, tag="scaleb")
                nc.vector.tensor_scalar(
                    scaleb[:], expb[:],
                    scalar1=(MBITS_M1 << 23), scalar2=-(MBITS_M1 << 23),
                    op0=mybir.AluOpType.max, op1=mybir.AluOpType.add,
                )
                rcpb = pool.tile([P, nb], mybir.dt.int32, tag="rcpb")
                nc.vector.tensor_scalar(
                    rcpb[:], scaleb[:], scalar1=-1, scalar2=(254 << 23),
                    op0=mybir.AluOpType.mult, op1=mybir.AluOpType.add,
                )
                rcp_b = rcpb[:].bitcast(mybir.dt.float32).unsqueeze(2).broadcast_to((P, nb, BLK))
                sc_b = scaleb[:].bitcast(mybir.dt.float32).unsqueeze(2).broadcast_to((P, nb, BLK))

                # q = sat_int8(round(x * rcp))
                q = pool.tile([P, f], mybir.dt.int8, tag="q")
                q3 = q[:].rearrange("p (b k) -> p b k", k=BLK)
                nc.vector.tensor_tensor(q3, x3, rcp_b, op=mybir.AluOpType.mult)

                # dequant on GPSIMD: f32 for the first ba blocks, bf16 for the rest
                d32 = pool.tile([P, fa], mybir.dt.float32, tag="d32")
                nc.gpsimd.tensor_tensor(
                    d32[:].rearrange("p (b k) -> p b k", k=BLK),
                    q3[:, :ba], sc_b[:, :ba], op=mybir.AluOpType.mult,
                )
                d16 = pool.tile([P, f - fa], mybir.dt.bfloat16, tag="d16")
                nc.gpsimd.tensor_tensor(
                    d16[:].rearrange("p (b k) -> p b k", k=BLK),
                    q3[:, ba:], sc_b[:, ba:], op=mybir.AluOpType.mult,
                )
                nc.scalar.dma_start(
                    out32[o32:o32 + P * fa].rearrange("(p f) -> p f", p=P), d32[:])
                nc.sync.dma_start(
                    out16[o16:o16 + P * (f - fa)].rearrange("(p f) -> p f", p=P), d16[:])
    nc.finalize()
    return nc


_NC_CACHE = {}


def _get_nc(reps=1):
    if reps not in _NC_CACHE:
        _NC_CACHE[reps] = build(reps)
    return _NC_CACHE[reps]


def kernel(x: np.ndarray) -> np.ndarray:
    x = np.asarray(x)
    assert x.shape == (N_CORES, ROWS, COLS) and x.dtype == np.float32, (x.shape, x.dtype)
    nc = _get_nc()
    in_maps = [{"x": np.ascontiguousarray(x[c])} for c in range(N_CORES)]
    res = run_bass_kernel_spmd(nc, in_maps, core_ids=list(range(N_CORES)))
    parts, _, _ = _splits()
    outs = []
    for r in res.results:
        a32 = np.asarray(r["out32"])
        a16 = np.asarray(r["out16"]).astype(np.float32)
        full = np.empty(ROWS * COLS, dtype=np.float32)
        for f, fa, o, o32, o16 in parts:
            tilef = np.empty((P, f), dtype=np.float32)
            tilef[:, :fa] = a32[o32:o32 + P * fa].reshape(P, fa)
            tilef[:, fa:] = a16[o16:o16 + P * (f - fa)].reshape(P, f - fa)
            full[o:o + P * f] = tilef.reshape(-1)
        outs.append(full.reshape(ROWS, COLS))
    return np.stack(outs, axis=0)


# revision 16
# speedup vs baseline: 1.1400x; 1.1400x over previous
"""BFP8 block quantize-dequantize for Trainium2 (Bass/Tile), 8-core data parallel.

Problem: x (8, 4096, 4096) f32. Each contiguous block of 16 elements (along the
flattened last dims) shares an exponent e = floor(log2(max|x|)); values are
quantized to signed 8-bit mantissas at scale 2^(e-7) and dequantized back.

Sharding: pure data parallel on the leading axis — core c processes x[c]
([4096, 4096] = 64 MiB in, 64 MiB out). No cross-core communication.

Per-core kernel (memory-bound; HBM roofline ~360 GB/s/core -> ~373 us):
  - 16 MiB-contiguous tiles [128 x 4096] f32, triple-plus buffered (bufs=4).
  - Loads issued from SP (sync) HWDGE, stores from ACT (scalar) HWDGE so the
    two directions ride separate queue sets and overlap.
  - VectorE: abs-max reduce over [128, 256, 16] -> block max; exponent bit-math
    (no log2/exp2 needed: for normal floats floor(log2(m)) is the exponent
    field, so scale = 2^(e-7) and rcp = 2^(7-e) are exact bit manipulations);
    quantize q = sat_int8(round(x * rcp)) — the f32->int8 output conversion
    gives round-to-nearest-even + clamp to [-128, 127] for free, which is
    exactly clip(round(.), qmin, qmax).
  - GpSimd: dequantize out = q * scale (int8 x f32-broadcast -> f32), keeping
    VectorE under the DMA roofline.
Zero/denormal blocks: expb clamps to 0 -> scale 0 -> out exactly 0.
"""
import numpy as np

try:
    import concourse.bacc as bacc
except ImportError:  # pragma: no cover - fallback for bare environments
    import sys
    for _p in ("/opt/trn_rl_repo", "/root/.axon_site/_ro/trn_rl_repo"):
        if _p not in sys.path:
            sys.path.insert(0, _p)
    import concourse.bacc as bacc
import concourse.mybir as mybir
import concourse.tile as tile
from concourse.bass_utils import run_bass_kernel_spmd

N_CORES = 8
P = 128                      # SBUF partitions
ROWS, COLS = 4096, 4096      # per-core shard
BLK = 16                     # elements sharing one exponent
MBITS_M1 = 7                 # mantissa_bits - 1
EXP_MASK = 0x7F800000

TILE_F = 4096                # f32 elements per partition per steady-state tile
TAPER_N, TAPER_F = 2, 1024   # smaller tiles at each end: faster pipeline fill/drain
BUFS = 4


def _schedule():
    total_f = ROWS * COLS // P
    end = TAPER_N * TAPER_F
    mid = total_f - 2 * end
    assert mid % TILE_F == 0
    return [TAPER_F] * TAPER_N + [TILE_F] * (mid // TILE_F) + [TAPER_F] * TAPER_N


def build(reps=1):
    nc = bacc.Bacc()
    x = nc.dram_tensor("x", [ROWS, COLS], mybir.dt.float32, kind="ExternalInput")
    out = nc.dram_tensor("out", [ROWS, COLS], mybir.dt.float32, kind="ExternalOutput")

    sched = _schedule()
    offs = [0]
    for f in sched:
        offs.append(offs[-1] + P * f)
    assert offs[-1] == ROWS * COLS
    xflat = x[:].rearrange("r c -> (r c)")
    outflat = out[:].rearrange("r c -> (r c)")

    with tile.TileContext(nc) as tc:
        with tc.tile_pool(name="sbuf", bufs=BUFS) as pool:
            for t, f in [(t, f) for _ in range(reps) for t, f in enumerate(sched)]:
                nb = f // BLK
                xt = pool.tile([P, f], mybir.dt.float32, tag="x")
                nc.sync.dma_start(xt[:], xflat[offs[t]:offs[t + 1]].rearrange("(p f) -> p f", p=P))
                x3 = xt[:].rearrange("p (b k) -> p b k", k=BLK)

                # block max|x|
                bmax = pool.tile([P, nb], mybir.dt.float32, tag="bmax")
                nc.vector.tensor_reduce(
                    bmax[:], x3, axis=mybir.AxisListType.X,
                    op=mybir.AluOpType.max, apply_absolute_value=True,
                )
                # expb = exponent field of bmax == bits of 2^e
                expb = pool.tile([P, nb], mybir.dt.int32, tag="expb")
                nc.vector.tensor_scalar(
                    expb[:], bmax[:].bitcast(mybir.dt.int32),
                    scalar1=EXP_MASK, scalar2=None,
                    op0=mybir.AluOpType.bitwise_and,
                )
                # scale_bits = max(expb, 7<<23) - (7<<23)   [= 2^(e-7); 0 for zero/denormal blocks]
                scaleb = pool.tile([P, nb], mybir.dt.int32, tag="scaleb")
                nc.vector.tensor_scalar(
                    scaleb[:], expb[:],
                    scalar1=(MBITS_M1 << 23), scalar2=-(MBITS_M1 << 23),
                    op0=mybir.AluOpType.max, op1=mybir.AluOpType.add,
                )
                # rcp_bits = (254<<23) - scale_bits         [= 2^(7-e)]
                rcpb = pool.tile([P, nb], mybir.dt.int32, tag="rcpb")
                nc.vector.tensor_scalar(
                    rcpb[:], scaleb[:], scalar1=-1, scalar2=(254 << 23),
                    op0=mybir.AluOpType.mult, op1=mybir.AluOpType.add,
                )
                scale_b = scaleb[:].bitcast(mybir.dt.float32).unsqueeze(2).broadcast_to((P, nb, BLK))
                rcp_b = rcpb[:].bitcast(mybir.dt.float32).unsqueeze(2).broadcast_to((P, nb, BLK))

                # q = sat_int8(round(x * rcp)) == clip(round(x / scale), -128, 127)
                q = pool.tile([P, f], mybir.dt.int8, tag="q")
                nc.vector.tensor_tensor(
                    q[:].rearrange("p (b k) -> p b k", k=BLK),
                    x3, rcp_b, op=mybir.AluOpType.mult,
                )
                # out = q * scale
                deq = pool.tile([P, f], mybir.dt.float32, tag="deq")
                nc.gpsimd.tensor_tensor(
                    deq[:].rearrange("p (b k) -> p b k", k=BLK),
                    q[:].rearrange("p (b k) -> p b k", k=BLK),
                    scale_b, op=mybir.AluOpType.mult,
                )
                nc.scalar.dma_start(
                    outflat[offs[t]:offs[t + 1]].rearrange("(p f) -> p f", p=P), deq[:])
    nc.finalize()
    return nc


_NC_CACHE = {}


def _get_nc(reps=1):
    if reps not in _NC_CACHE:
        _NC_CACHE[reps] = build(reps)
    return _NC_CACHE[reps]


def kernel(x: np.ndarray) -> np.ndarray:
    x = np.asarray(x)
    assert x.shape == (N_CORES, ROWS, COLS) and x.dtype == np.float32, (x.shape, x.dtype)
    nc = _get_nc()
    in_maps = [{"x": np.ascontiguousarray(x[c])} for c in range(N_CORES)]
    res = run_bass_kernel_spmd(nc, in_maps, core_ids=list(range(N_CORES)))
    return np.stack([r["out"] for r in res.results], axis=0)

